# revision 35
# baseline (speedup 1.0000x reference)
"""Trainium2 Bass kernel for KeyValueAttention (4-head masked attention, gated combine).

v4 strategy (8 NeuronCores, query-dim sharded, 512 queries/core):
  Chunk-major schedule in TWO passes (heads {0,1} then {2,3}) so the two
  score matmuls of a chunk share the keys stationary load.
  - psum banks: scores h0 (3 rot) + scores h1 (2 rot) + AV x2 + vbuild misc = 8.
  - Masked exp per (chunk, head) slot, three balanced paths:
      'C'  (DVE), 'Bd' (PE mask pre-add + ACT exp bias -20), 'Bp' (ACT+Pool).
  - V built per 4-chunk group, prefetch 8 chunks, single misc bank.
  - Builds and head-0/1 output projection dripped into the chunk stream.

Measured: 178716 ns, rel err 0.0036.
"""

import os
import numpy as np

NQ, NK, DC, A, H, DO = 4096, 8192, 256, 64, 4, 256
NCORES = 8
NQC = NQ // NCORES
KC = 128
NKC = NK // KC
NPAIR = NKC // 2

TYPE_PATTERN = [
    ("C", "Bp"),
    ("Bp", "C"),
    ("C", "Bd"),
    ("Bp", "C"),
    ("C", "Bd"),
    ("Bd", "C"),
    ("C", "Bp"),
    ("Bp", "C"),
]

_cache = {}


def _fit_exp_poly(scale=0.125, lo=-0.85, hi=0.85):
    t = np.linspace(lo, hi, 40001)
    w = 1.0 / np.exp(t)
    Amat = np.stack([t, t * t, t ** 3], axis=1) * w[:, None]
    a = np.linalg.lstsq(Amat, (np.exp(t) - 1.0) * w, rcond=None)[0]
    return [float(a[0] * scale), float(a[1] * scale ** 2), float(a[2] * scale ** 3)]


POLY_B = _fit_exp_poly()


def _register_dve_exp_op():
    from concourse.dve_spec import Spec, Src0, Src1, C0, C1, C2, One, lower
    from concourse.dve_ops import (
        DveOp, OPS, CUSTOM_DVE_SPECS, _SUB_OPCODE_FOR_NAME, _CUSTOM_DVE_ROW_BASE,
    )
    from concourse.dve_table_gen import dve_ver_for
    from concourse.dve_uop import DveOpSpec

    name = "EXP_POLY_MASK_ANT"
    if name in _SUB_OPCODE_FOR_NAME:
        return next(op for op in OPS if op.name == name)

    body = (((Src0 * C2 + C1) * Src0 + C0) * Src0 + One) * Src1
    spec = Spec(
        body=body,
        reference=lambda in0, in1, s0, s1, imm2: (
            (((in0 * imm2 + s1) * in0 + s0) * in0 + 1.0) * in1
        ),
    )
    op = DveOp(name, spec, subdim=False, uops_sha={})
    ver = dve_ver_for("TRN2")
    op.uops_sha[ver] = DveOpSpec(
        name=name, opcode=31, uops=lower(spec, ver=ver), rd1_en=True
    ).sha(ver)
    OPS.append(op)
    CUSTOM_DVE_SPECS[name] = spec
    _SUB_OPCODE_FOR_NAME[name] = _CUSTOM_DVE_ROW_BASE + len(OPS) - 1
    return op


def _build_kernel():
    import concourse.bacc as bacc
    import concourse.mybir as mybir
    from concourse.tile import TileContext

    EXP_OP = _register_dve_exp_op()

    F32 = mybir.dt.float32
    BF16 = mybir.dt.bfloat16
    FP8 = mybir.dt.float8e4
    AF = mybir.ActivationFunctionType
    ALU = mybir.AluOpType
    DR = mybir.MatmulPerfMode.DoubleRow

    nc = bacc.Bacc(None, target_bir_lowering=False, debug=False)

    xqtb = nc.dram_tensor("xqtb", [128, 2, NQC], BF16, kind="ExternalInput")
    xkt8 = nc.dram_tensor("xkt8", [128, 2, NK], FP8, kind="ExternalInput")
    wqTb = nc.dram_tensor("wqTb", [64, 2, H, 128], BF16, kind="ExternalInput")
    wkTb = nc.dram_tensor("wkTb", [64, 2, H, 128], BF16, kind="ExternalInput")
    wv8 = nc.dram_tensor("wv8", [128, 2, H * A], FP8, kind="ExternalInput")
    wgtb = nc.dram_tensor("wgtb", [128, 2, H], BF16, kind="ExternalInput")
    bg = nc.dram_tensor("bg", [H, 1], F32, kind="ExternalInput")
    wo = nc.dram_tensor("wo", [A, DO], F32, kind="ExternalInput")
    bo = nc.dram_tensor("bo", [1, DO], F32, kind="ExternalInput")
    i80d = nc.dram_tensor("i80d", [128, 2, 128], FP8, kind="ExternalInput")
    i4 = nc.dram_tensor("i4", [H, H], F32, kind="ExternalInput")
    maskx = nc.dram_tensor("maskx", [NKC, 128, NQC], FP8, kind="ExternalInput")
    out = nc.dram_tensor("out", [NQC, DO], F32, kind="ExternalOutput")

    with TileContext(nc) as tc:
        with tc.sbuf_pool(name="consts", bufs=1) as cpool:
            # scalar (ACT hwdge) queue stays SHORT so early psum->sbuf copies
            # on the scalar engine are not stuck behind DMA issues.
            xqtb_t = cpool.tile([128, 2, NQC], BF16)
            nc.scalar.dma_start(xqtb_t, xqtb[:])
            xkt_t = cpool.tile([128, 2, NK], FP8)
            nc.scalar.dma_start(xkt_t[:, :, 0:1024], xkt8[:, :, 0:1024])
            wv_t = cpool.tile([128, 2, H * A], FP8)
            nc.scalar.dma_start(wv_t, wv8[:])
            i80_t = cpool.tile([128, 2, 128], FP8)
            nc.scalar.dma_start(i80_t, i80d[:])
            wgt_t = cpool.tile([128, 2, H], BF16)
            nc.scalar.dma_start(wgt_t, wgtb[:])
            bg_t = cpool.tile([H, 1], F32)
            nc.scalar.dma_start(bg_t, bg[:])
            i4_t = cpool.tile([H, H], F32)
            nc.scalar.dma_start(i4_t, i4[:])
            # sync queue: W2 weight operands first, then interleaved
            # mask batches / xkt remainder in consumption order.
            wqT_t = cpool.tile([64, 2, H, 128], BF16)
            nc.sync.dma_start(wqT_t, wqTb[:])
            wkT_t = cpool.tile([64, 2, H, 128], BF16)
            nc.sync.dma_start(wkT_t, wkTb[:])
            wo_t = cpool.tile([A, DO], F32)
            nc.sync.dma_start(wo_t, wo[:])
            bo_t = cpool.tile([1, DO], F32)
            nc.sync.dma_start(bo_t, bo[:])
            mask_sb = cpool.tile([128, NKC, NQC], FP8)

            def mask_batch(lo, hi):
                nc.sync.dma_start(
                    mask_sb[:, lo:hi, :],
                    maskx[lo:hi].rearrange("c p q -> p c q"),
                )

            mask_batch(0, 4)
            nc.sync.dma_start(xkt_t[:, :, 1024:4096], xkt8[:, :, 1024:4096])
            mask_batch(4, 12)
            nc.sync.dma_start(xkt_t[:, :, 4096:NK], xkt8[:, :, 4096:NK])
            for lo, hi in [(12, 20), (20, 28), (28, 36), (36, 44),
                           (44, 52), (52, 64)]:
                mask_batch(lo, hi)

            negb = cpool.tile([128, 1], F32)
            nc.vector.memset(negb, -20.0)
            woaug = cpool.tile([A + 1, DO + 1], BF16)
            ones1 = cpool.tile([1, 128], F32)

            qw8 = [cpool.tile([128, 2, NQC], FP8, name=f"qw{h}") for h in range(H)]
            # W2_h = Wk_h @ Wq_h^T in bf16, layout [c' pair part, i, h, c]
            w2sb = cpool.tile([128, 2, H, DC], BF16)
            vaug = cpool.tile([128, H, NKC, 80], FP8)
            nc.gpsimd.memset(vaug[:, :, :, A : A + 1], 1.0)
            gates = cpool.tile([H, NQC], F32)
            gt_sb = cpool.tile([128, 4 * H], F32)
            boB_sb = cpool.tile([128, DO], F32)
            nh = [cpool.tile([A + 1, NQC], BF16, name=f"nh{h}") for h in range(H)]
            acc_a = [cpool.tile([128, DO], F32, name=f"acca{q}") for q in range(4)]
            acc_b = [cpool.tile([128, DO], F32, name=f"accb{q}") for q in range(4)]

            with (
                tc.psum_pool(name="pm", bufs=1) as pm,
                tc.sbuf_pool(name="ms", bufs=1) as ms,
            ):
                def s_tile(hi):
                    return pm.tile([128, NQC], F32, tag=f"s{hi}",
                                   bufs=3 if hi == 0 else 2, name=f"s{hi}")

                def build_w2(h):
                    # W2_h[c', c] = sum_a Wq[c', a] Wk[c, a]  (weights only --
                    # runs before x_Q even lands)
                    for i in range(2):
                        ps = s_tile(i)
                        nc.tensor.matmul(
                            ps[:, :DC].rearrange("p (j m) -> p j m", j=2),
                            wqT_t[:, i, h, :],
                            wkT_t[:, :, h, :],
                            start=True, stop=True,
                        )
                        nc.scalar.copy(w2sb[:, i, h, :], ps[:, :DC])

                def build_qw(h):
                    # QW_h = W2_h @ x_Q^T -> fp8 [128, 2, NQC]
                    for half in range(2):
                        qps = s_tile(half)
                        for i in range(2):
                            nc.tensor.matmul(
                                qps,
                                w2sb[:, i, h, half * 128 : (half + 1) * 128],
                                xqtb_t[:, i, :],
                                start=(i == 0), stop=(i == 1),
                            )
                        if half == 0:
                            nc.scalar.copy(qw8[h][:, half, :], qps)
                        else:
                            nc.vector.tensor_copy(qw8[h][:, half, :], qps)

                def build_woaug():
                    nc.vector.memset(woaug, 0.0)
                    nc.vector.tensor_copy(woaug[:A, :DO], wo_t)
                    nc.vector.memset(woaug[A : A + 1, DO : DO + 1], 1.0)
                    nc.vector.memset(ones1, 1.0)

                def build_gates():
                    g_ps = s_tile(1)
                    for i in range(2):
                        nc.tensor.matmul(
                            g_ps[0:4, :], wgt_t[:, i, :], xqtb_t[:, i, :],
                            start=(i == 0), stop=(i == 1),
                        )
                    nc.scalar.activation(gates, g_ps[0:4, :], AF.Sigmoid,
                                         bias=bg_t[:], scale=1.0)

                def build_gt():
                    gt_ps = s_tile(1)
                    for q in range(4):
                        nc.tensor.transpose(
                            gt_ps[:, q * H : q * H + H],
                            gates[:, q * 128 : (q + 1) * 128],
                            i4_t[:],
                        )
                    nc.vector.tensor_copy(gt_sb, gt_ps[:, : 4 * H])

                def build_boB():
                    boB_ps = s_tile(1)
                    nc.tensor.matmul(boB_ps[:, :DO], ones1, bo_t,
                                     start=True, stop=True)
                    nc.vector.tensor_copy(boB_sb, boB_ps[:, :DO])

                def vbuild_group(t):
                    P, c0 = t // 64, t % 64
                    vg = pm.tile([128, 4, 2 * A], F32, tag="vb", bufs=1,
                                 name="vg")
                    for s in range(4):
                        c = c0 + s
                        nc.tensor.matmul(
                            vg[:, s, :],
                            xkt_t[:, :, c * KC : (c + 1) * KC],
                            wv_t[:, :, 2 * P * A : (2 * P + 2) * A],
                            start=True, stop=True, perf_mode=DR,
                        )
                    src = vg.rearrange("p s (h a) -> p s h a", h=2)
                    dst = vaug[:, 2 * P : 2 * P + 2, c0 : c0 + 4, 0:A]
                    nc.vector.tensor_copy(dst.rearrange("p h s a -> p s h a"),
                                          src)

                def epilogue_head(h, q):
                    p_ps = s_tile(0)
                    nc.tensor.matmul(
                        p_ps[:, : DO + 1],
                        nh[h][:, q * 128 : (q + 1) * 128],
                        woaug,
                        start=True, stop=True,
                    )
                    rden = ms.tile([128, 1], F32, tag="rden", bufs=2,
                                   name="rden")
                    nc.vector.reciprocal(rden, p_ps[:, DO : DO + 1])
                    sc = ms.tile([128, 1], F32, tag="sc", bufs=2, name="sc")
                    nc.vector.tensor_mul(
                        sc, rden, gt_sb[:, q * H + h : q * H + h + 1]
                    )
                    prev = boB_sb if h == 0 else (acc_a[q] if h % 2 == 1 else acc_b[q])
                    dst = acc_a[q] if h % 2 == 0 else acc_b[q]
                    nc.vector.scalar_tensor_tensor(
                        dst, p_ps[:, :DO], sc, prev,
                        op0=ALU.mult, op1=ALU.add,
                    )
                    if h == H - 1:
                        nc.sync.dma_start(
                            out[q * 128 : (q + 1) * 128, :], dst
                        )

                build_w2(0)
                build_qw(0)
                build_w2(1)
                build_qw(1)

                drip = {
                    1: lambda: vbuild_group(0),
                    2: lambda: vbuild_group(4),
                    5: lambda: build_w2(2),
                    7: lambda: build_qw(2),
                    11: lambda: build_w2(3),
                    15: lambda: build_qw(3),
                    19: build_gates,
                    23: build_gt,
                    27: build_woaug,
                    31: build_boB,
                }
                for i, (h, q) in enumerate([(hh, qq) for hh in (0, 1)
                                            for qq in range(4)]):
                    drip[64 + 5 + 7 * i] = (
                        lambda h=h, q=q: epilogue_head(h, q)
                    )

                for P in range(2):
                    ha, hb = 2 * P, 2 * P + 1
                    av = [
                        pm.tile([A + 1, NQC], F32, tag=f"av{hi}", bufs=1,
                                name=f"av{hi}")
                        for hi in range(2)
                    ]
                    pend = []

                    def emit_av(pair, ems):
                        for hi in range(2):
                            nc.tensor.matmul(
                                av[hi],
                                vaug[:, 2 * P + hi, 2 * pair : 2 * pair + 2,
                                     0 : A + 1],
                                ems[hi],
                                start=(pair == 0), stop=(pair == NPAIR - 1),
                                perf_mode=DR,
                            )

                    em_cur = [None, None]
                    for c in range(NKC):
                        t = P * 64 + c
                        pair, slot = divmod(c, 2)
                        if t % 4 == 0 and t + 8 < 128:
                            vbuild_group(t + 8)
                        if t in drip:
                            drip[t]()

                        tys = TYPE_PATTERN[c % 8]
                        s = [None, None]
                        for hi in range(2):
                            s[hi] = s_tile(hi)
                            if tys[hi] == "Bd":
                                nc.tensor.matmul(
                                    s[hi], i80_t,
                                    mask_sb[:, c : c + 1, :].broadcast_to(
                                        (128, 2, NQC)),
                                    start=True, stop=False, perf_mode=DR,
                                )
                        for hi, h in enumerate((ha, hb)):
                            nc.tensor.matmul(
                                s[hi],
                                xkt_t[:, :, c * KC : (c + 1) * KC],
                                qw8[h],
                                start=(tys[hi] != "Bd"), stop=True,
                                perf_mode=DR,
                            )
                        for hi in range(2):
                            if slot == 0:
                                em_cur[hi] = ms.tile([128, 2, NQC], FP8,
                                                     tag=f"em{hi}", bufs=4,
                                                     name=f"em{hi}")
                            dst = em_cur[hi][:, slot]
                            ty = tys[hi]
                            if ty == "C":
                                nc.vector._custom_dve(
                                    EXP_OP, out=dst, in0=s[hi],
                                    in1=mask_sb[:, c, :],
                                    s0=POLY_B[0], s1=POLY_B[1], imm2=POLY_B[2],
                                )
                            elif ty == "Bd":
                                nc.scalar.activation(
                                    dst, s[hi], AF.Exp, bias=negb[:], scale=0.125
                                )
                            else:
                                nc.scalar.activation(
                                    dst, s[hi], AF.Exp, bias=0.0, scale=0.125
                                )
                                nc.gpsimd.tensor_mul(dst, dst,
                                                     mask_sb[:, c, :])
                        if slot == 1:
                            pend.append((pair, list(em_cur)))
                            if len(pend) > 2:
                                emit_av(*pend.pop(0))
                    for item in pend:
                        emit_av(*item)

                    nc.scalar.copy(nh[ha], av[0])
                    nc.vector.tensor_copy(nh[hb], av[1])

                for h in (2, 3):
                    for q in range(4):
                        epilogue_head(h, q)
    nc.finalize()
    return nc


def _to_f8(x):
    import ml_dtypes
    return np.ascontiguousarray(np.asarray(x, dtype=np.float32).astype(
        ml_dtypes.float8_e4m3fn))


def _to_bf16(x):
    import ml_dtypes
    return np.ascontiguousarray(np.asarray(x, dtype=np.float32).astype(
        ml_dtypes.bfloat16))


def _dr_c_layout(xT):
    return np.ascontiguousarray(xT.reshape(2, 128, -1).transpose(1, 0, 2))


def _prep_shared(x_K, Wq, Wk, Wv, Wg, bg, Wo, bo):
    xkt = x_K.T
    xkt8 = _to_f8(_dr_c_layout(xkt))
    wqTb = _to_bf16(Wq.reshape(H, 2, 128, A).transpose(3, 1, 0, 2))
    wkTb = _to_bf16(Wk.reshape(H, 2, 128, A).transpose(3, 1, 0, 2))
    arr = np.empty((128, 2, H * A), np.float32)
    for h in range(H):
        arr[:, :, h * A:(h + 1) * A] = Wv[h].reshape(2, 128, A).transpose(1, 0, 2)
    wv8 = _to_f8(arr)
    wgtb = _to_bf16(Wg.T.reshape(2, 128, H).transpose(1, 0, 2))
    i80d = np.zeros((128, 2, 128), np.float32)
    for p in range(128):
        i80d[p, :, p] = 80.0
    return {
        "xkt8": xkt8, "wqTb": wqTb, "wkTb": wkTb, "wv8": wv8, "wgtb": wgtb,
        "bg": np.asarray(bg, np.float32).reshape(H, 1),
        "wo": np.ascontiguousarray(np.asarray(Wo, np.float32)),
        "bo": np.asarray(bo, np.float32).reshape(1, DO),
        "i80d": _to_f8(i80d),
        "i4": np.eye(H, dtype=np.float32),
    }


def _prep_mask_core(mask_sl):
    import ml_dtypes
    mt = mask_sl.T.astype(np.float32)
    m3 = mt.reshape(NKC, KC, NQC)
    return np.ascontiguousarray(m3.astype(ml_dtypes.float8_e4m3fn))


def kernel(x_Q, x_K, mask, Wq, Wk, Wv, Wg, bg, Wo, bo):
    from concourse.bass_utils import run_bass_kernel_spmd

    x_Q = np.asarray(x_Q, dtype=np.float32)
    x_K = np.asarray(x_K, dtype=np.float32)
    mask = np.asarray(mask, dtype=np.int32)

    shared = _prep_shared(
        x_K, np.asarray(Wq, np.float32), np.asarray(Wk, np.float32),
        np.asarray(Wv, np.float32), np.asarray(Wg, np.float32),
        bg, Wo, bo,
    )

    in_maps = []
    for cidx in range(NCORES):
        sl = slice(cidx * NQC, (cidx + 1) * NQC)
        xqt = x_Q[sl].T
        m = {
            "xqtb": _to_bf16(_dr_c_layout(xqt)),
            "maskx": _prep_mask_core(mask[sl]),
        }
        m.update(shared)
        in_maps.append(m)

    if "nc" not in _cache:
        _cache["nc"] = _build_kernel()
    res = run_bass_kernel_spmd(
        _cache["nc"], in_maps, list(range(NCORES)),
        trace=bool(int(os.environ.get("BASS_KERNEL_TRACE", "0"))),
    )
    if res.exec_time_ns is not None:
        print(f"HW exec time: {res.exec_time_ns} ns")
    return np.concatenate([r["out"] for r in res.results], axis=0)


# revision 36
# speedup vs baseline: 1.0276x; 1.0276x over previous
"""Trainium2 Bass kernel for KeyValueAttention (4-head masked attention, gated combine).

v4 strategy (8 NeuronCores, query-dim sharded, 512 queries/core):
  Chunk-major schedule in TWO passes (heads {0,1} then {2,3}) so the two
  score matmuls of a chunk share the keys stationary load.
  - psum banks: scores h0 (3 rot) + scores h1 (2 rot) + AV x2 + vbuild misc = 8.
  - Masked exp per (chunk, head) slot, three balanced paths:
      'C'  (DVE), 'Bd' (PE mask pre-add + ACT exp bias -20), 'Bp' (ACT+Pool).
  - V built per 4-chunk group, prefetch 8 chunks, single misc bank.
  - Builds and head-0/1 output projection dripped into the chunk stream.

Measured: 178716 ns, rel err 0.0036.
"""

import os
import numpy as np

NQ, NK, DC, A, H, DO = 4096, 8192, 256, 64, 4, 256
NCORES = 8
NQC = NQ // NCORES
KC = 128
NKC = NK // KC
NPAIR = NKC // 2

TYPE_PATTERN = [
    ("C", "Bp"),
    ("Bp", "C"),
    ("C", "Bd"),
    ("Bp", "C"),
    ("C", "Bp"),
    ("Bd", "C"),
    ("C", "Bp"),
    ("Bp", "C"),
]

_cache = {}


def _fit_exp_poly(scale=0.125, lo=-0.85, hi=0.85):
    t = np.linspace(lo, hi, 40001)
    w = 1.0 / np.exp(t)
    Amat = np.stack([t, t * t, t ** 3], axis=1) * w[:, None]
    a = np.linalg.lstsq(Amat, (np.exp(t) - 1.0) * w, rcond=None)[0]
    return [float(a[0] * scale), float(a[1] * scale ** 2), float(a[2] * scale ** 3)]


POLY_B = _fit_exp_poly()


def _register_dve_exp_op():
    from concourse.dve_spec import Spec, Src0, Src1, C0, C1, C2, One, lower
    from concourse.dve_ops import (
        DveOp, OPS, CUSTOM_DVE_SPECS, _SUB_OPCODE_FOR_NAME, _CUSTOM_DVE_ROW_BASE,
    )
    from concourse.dve_table_gen import dve_ver_for
    from concourse.dve_uop import DveOpSpec

    name = "EXP_POLY_MASK_ANT"
    if name in _SUB_OPCODE_FOR_NAME:
        return next(op for op in OPS if op.name == name)

    body = (((Src0 * C2 + C1) * Src0 + C0) * Src0 + One) * Src1
    spec = Spec(
        body=body,
        reference=lambda in0, in1, s0, s1, imm2: (
            (((in0 * imm2 + s1) * in0 + s0) * in0 + 1.0) * in1
        ),
    )
    op = DveOp(name, spec, subdim=False, uops_sha={})
    ver = dve_ver_for("TRN2")
    op.uops_sha[ver] = DveOpSpec(
        name=name, opcode=31, uops=lower(spec, ver=ver), rd1_en=True
    ).sha(ver)
    OPS.append(op)
    CUSTOM_DVE_SPECS[name] = spec
    _SUB_OPCODE_FOR_NAME[name] = _CUSTOM_DVE_ROW_BASE + len(OPS) - 1
    return op


def _build_kernel():
    import concourse.bacc as bacc
    import concourse.mybir as mybir
    from concourse.tile import TileContext

    EXP_OP = _register_dve_exp_op()

    F32 = mybir.dt.float32
    BF16 = mybir.dt.bfloat16
    FP8 = mybir.dt.float8e4
    AF = mybir.ActivationFunctionType
    ALU = mybir.AluOpType
    DR = mybir.MatmulPerfMode.DoubleRow

    nc = bacc.Bacc(None, target_bir_lowering=False, debug=False)

    xqtb = nc.dram_tensor("xqtb", [128, 2, NQC], BF16, kind="ExternalInput")
    xkt8 = nc.dram_tensor("xkt8", [128, 2, NK], FP8, kind="ExternalInput")
    wqTb = nc.dram_tensor("wqTb", [64, 2, H, 128], BF16, kind="ExternalInput")
    wkTb = nc.dram_tensor("wkTb", [64, 2, H, 128], BF16, kind="ExternalInput")
    wv8 = nc.dram_tensor("wv8", [128, 2, H * A], FP8, kind="ExternalInput")
    wgtb = nc.dram_tensor("wgtb", [128, 2, H], BF16, kind="ExternalInput")
    bg = nc.dram_tensor("bg", [H, 1], F32, kind="ExternalInput")
    wo = nc.dram_tensor("wo", [A, DO], F32, kind="ExternalInput")
    bo = nc.dram_tensor("bo", [1, DO], F32, kind="ExternalInput")
    i80d = nc.dram_tensor("i80d", [128, 2, 128], FP8, kind="ExternalInput")
    i4 = nc.dram_tensor("i4", [H, H], F32, kind="ExternalInput")
    maskx = nc.dram_tensor("maskx", [NKC, 128, NQC], FP8, kind="ExternalInput")
    out = nc.dram_tensor("out", [NQC, DO], F32, kind="ExternalOutput")

    with TileContext(nc) as tc:
        with tc.sbuf_pool(name="consts", bufs=1) as cpool:
            # scalar (ACT hwdge) queue stays SHORT so early psum->sbuf copies
            # on the scalar engine are not stuck behind DMA issues.
            xqtb_t = cpool.tile([128, 2, NQC], BF16)
            nc.scalar.dma_start(xqtb_t, xqtb[:])
            xkt_t = cpool.tile([128, 2, NK], FP8)
            nc.scalar.dma_start(xkt_t[:, :, 0:1024], xkt8[:, :, 0:1024])
            wv_t = cpool.tile([128, 2, H * A], FP8)
            nc.scalar.dma_start(wv_t, wv8[:])
            i80_t = cpool.tile([128, 2, 128], FP8)
            nc.scalar.dma_start(i80_t, i80d[:])
            wgt_t = cpool.tile([128, 2, H], BF16)
            nc.scalar.dma_start(wgt_t, wgtb[:])
            bg_t = cpool.tile([H, 1], F32)
            nc.scalar.dma_start(bg_t, bg[:])
            i4_t = cpool.tile([H, H], F32)
            nc.scalar.dma_start(i4_t, i4[:])
            # sync queue: W2 weight operands first, then interleaved
            # mask batches / xkt remainder in consumption order.
            wqT_t = cpool.tile([64, 2, H, 128], BF16)
            nc.sync.dma_start(wqT_t, wqTb[:])
            wkT_t = cpool.tile([64, 2, H, 128], BF16)
            nc.sync.dma_start(wkT_t, wkTb[:])
            wo_t = cpool.tile([A, DO], F32)
            nc.sync.dma_start(wo_t, wo[:])
            bo_t = cpool.tile([1, DO], F32)
            nc.sync.dma_start(bo_t, bo[:])
            mask_sb = cpool.tile([128, NKC, NQC], FP8)

            def mask_batch(lo, hi):
                nc.sync.dma_start(
                    mask_sb[:, lo:hi, :],
                    maskx[lo:hi].rearrange("c p q -> p c q"),
                )

            mask_batch(0, 4)
            nc.sync.dma_start(xkt_t[:, :, 1024:4096], xkt8[:, :, 1024:4096])
            mask_batch(4, 12)
            nc.sync.dma_start(xkt_t[:, :, 4096:NK], xkt8[:, :, 4096:NK])
            for lo, hi in [(12, 20), (20, 28), (28, 36), (36, 44),
                           (44, 52), (52, 64)]:
                mask_batch(lo, hi)

            negb = cpool.tile([128, 1], F32)
            nc.vector.memset(negb, -20.0)
            woaug = cpool.tile([A + 1, DO + 1], BF16)
            ones1 = cpool.tile([1, 128], F32)

            qw8 = [cpool.tile([128, 2, NQC], FP8, name=f"qw{h}") for h in range(H)]
            # W2_h = Wk_h @ Wq_h^T in bf16, layout [c' pair part, i, h, c]
            w2sb = cpool.tile([128, 2, H, DC], BF16)
            vaug = cpool.tile([128, H, NKC, 80], FP8)
            nc.gpsimd.memset(vaug[:, :, :, A : A + 1], 1.0)
            gates = cpool.tile([H, NQC], F32)
            gt_sb = cpool.tile([128, 4 * H], F32)
            boB_sb = cpool.tile([128, DO], F32)
            nh = [cpool.tile([A + 1, NQC], BF16, name=f"nh{h}") for h in range(H)]
            acc_a = [cpool.tile([128, DO], F32, name=f"acca{q}") for q in range(4)]
            acc_b = [cpool.tile([128, DO], F32, name=f"accb{q}") for q in range(4)]

            with (
                tc.psum_pool(name="pm", bufs=1) as pm,
                tc.sbuf_pool(name="ms", bufs=1) as ms,
            ):
                def s_tile(hi):
                    return pm.tile([128, NQC], F32, tag=f"s{hi}",
                                   bufs=3 if hi == 0 else 2, name=f"s{hi}")

                def build_w2(h):
                    # W2_h[c', c] = sum_a Wq[c', a] Wk[c, a]  (weights only --
                    # runs before x_Q even lands)
                    for i in range(2):
                        ps = s_tile(i)
                        nc.tensor.matmul(
                            ps[:, :DC].rearrange("p (j m) -> p j m", j=2),
                            wqT_t[:, i, h, :],
                            wkT_t[:, :, h, :],
                            start=True, stop=True,
                        )
                        nc.scalar.copy(w2sb[:, i, h, :], ps[:, :DC])

                def build_qw(h):
                    # QW_h = W2_h @ x_Q^T -> fp8 [128, 2, NQC]
                    for half in range(2):
                        qps = s_tile(half)
                        for i in range(2):
                            nc.tensor.matmul(
                                qps,
                                w2sb[:, i, h, half * 128 : (half + 1) * 128],
                                xqtb_t[:, i, :],
                                start=(i == 0), stop=(i == 1),
                            )
                        if half == 0:
                            nc.scalar.copy(qw8[h][:, half, :], qps)
                        else:
                            nc.vector.tensor_copy(qw8[h][:, half, :], qps)

                def build_woaug():
                    nc.vector.memset(woaug, 0.0)
                    nc.vector.tensor_copy(woaug[:A, :DO], wo_t)
                    nc.vector.memset(woaug[A : A + 1, DO : DO + 1], 1.0)
                    nc.vector.memset(ones1, 1.0)

                def build_gates():
                    g_ps = s_tile(1)
                    for i in range(2):
                        nc.tensor.matmul(
                            g_ps[0:4, :], wgt_t[:, i, :], xqtb_t[:, i, :],
                            start=(i == 0), stop=(i == 1),
                        )
                    nc.scalar.activation(gates, g_ps[0:4, :], AF.Sigmoid,
                                         bias=bg_t[:], scale=1.0)

                def build_gt():
                    gt_ps = s_tile(1)
                    for q in range(4):
                        nc.tensor.transpose(
                            gt_ps[:, q * H : q * H + H],
                            gates[:, q * 128 : (q + 1) * 128],
                            i4_t[:],
                        )
                    nc.vector.tensor_copy(gt_sb, gt_ps[:, : 4 * H])

                def build_boB():
                    boB_ps = s_tile(1)
                    nc.tensor.matmul(boB_ps[:, :DO], ones1, bo_t,
                                     start=True, stop=True)
                    nc.vector.tensor_copy(boB_sb, boB_ps[:, :DO])

                def vbuild_group(t):
                    P, c0 = t // 64, t % 64
                    vg = pm.tile([128, 4, 2 * A], F32, tag="vb", bufs=1,
                                 name="vg")
                    for s in range(4):
                        c = c0 + s
                        nc.tensor.matmul(
                            vg[:, s, :],
                            xkt_t[:, :, c * KC : (c + 1) * KC],
                            wv_t[:, :, 2 * P * A : (2 * P + 2) * A],
                            start=True, stop=True, perf_mode=DR,
                        )
                    eng = (nc.scalar, nc.vector)[(t // 4) % 2]
                    src = vg.rearrange("p s (h a) -> p s h a", h=2)
                    dst = vaug[:, 2 * P : 2 * P + 2, c0 : c0 + 4, 0:A]
                    if eng is nc.scalar:
                        nc.scalar.copy(dst.rearrange("p h s a -> p s h a"), src)
                    else:
                        eng.tensor_copy(dst.rearrange("p h s a -> p s h a"), src)

                def epilogue_head(h, q):
                    p_ps = s_tile(0)
                    nc.tensor.matmul(
                        p_ps[:, : DO + 1],
                        nh[h][:, q * 128 : (q + 1) * 128],
                        woaug,
                        start=True, stop=True,
                    )
                    rden = ms.tile([128, 1], F32, tag="rden", bufs=2,
                                   name="rden")
                    nc.vector.reciprocal(rden, p_ps[:, DO : DO + 1])
                    sc = ms.tile([128, 1], F32, tag="sc", bufs=2, name="sc")
                    nc.vector.tensor_mul(
                        sc, rden, gt_sb[:, q * H + h : q * H + h + 1]
                    )
                    prev = boB_sb if h == 0 else (acc_a[q] if h % 2 == 1 else acc_b[q])
                    dst = acc_a[q] if h % 2 == 0 else acc_b[q]
                    nc.vector.scalar_tensor_tensor(
                        dst, p_ps[:, :DO], sc, prev,
                        op0=ALU.mult, op1=ALU.add,
                    )
                    if h == H - 1:
                        nc.sync.dma_start(
                            out[q * 128 : (q + 1) * 128, :], dst
                        )

                build_w2(0)
                build_qw(0)
                build_w2(1)
                build_qw(1)

                drip = {
                    1: lambda: vbuild_group(0),
                    2: lambda: vbuild_group(4),
                    5: lambda: build_w2(2),
                    7: lambda: build_qw(2),
                    11: lambda: build_w2(3),
                    15: lambda: build_qw(3),
                    19: build_gates,
                    23: build_gt,
                    27: build_woaug,
                    31: build_boB,
                }
                for i, (h, q) in enumerate([(hh, qq) for hh in (0, 1)
                                            for qq in range(4)]):
                    drip[64 + 5 + 7 * i] = (
                        lambda h=h, q=q: epilogue_head(h, q)
                    )

                for P in range(2):
                    ha, hb = 2 * P, 2 * P + 1
                    av = [
                        pm.tile([A + 1, NQC], F32, tag=f"av{hi}", bufs=1,
                                name=f"av{hi}")
                        for hi in range(2)
                    ]
                    pend = []

                    def emit_av(pair, ems):
                        for hi in range(2):
                            nc.tensor.matmul(
                                av[hi],
                                vaug[:, 2 * P + hi, 2 * pair : 2 * pair + 2,
                                     0 : A + 1],
                                ems[hi],
                                start=(pair == 0), stop=(pair == NPAIR - 1),
                                perf_mode=DR,
                            )

                    em_cur = [None, None]
                    for c in range(NKC):
                        t = P * 64 + c
                        pair, slot = divmod(c, 2)
                        if t % 4 == 0 and t + 8 < 128:
                            vbuild_group(t + 8)
                        if t in drip:
                            drip[t]()

                        tys = TYPE_PATTERN[c % 8]
                        s = [None, None]
                        for hi in range(2):
                            s[hi] = s_tile(hi)
                            if tys[hi] == "Bd":
                                nc.tensor.matmul(
                                    s[hi], i80_t,
                                    mask_sb[:, c : c + 1, :].broadcast_to(
                                        (128, 2, NQC)),
                                    start=True, stop=False, perf_mode=DR,
                                )
                        for hi, h in enumerate((ha, hb)):
                            nc.tensor.matmul(
                                s[hi],
                                xkt_t[:, :, c * KC : (c + 1) * KC],
                                qw8[h],
                                start=(tys[hi] != "Bd"), stop=True,
                                perf_mode=DR,
                            )
                        for hi in range(2):
                            if slot == 0:
                                em_cur[hi] = ms.tile([128, 2, NQC], FP8,
                                                     tag=f"em{hi}", bufs=4,
                                                     name=f"em{hi}")
                            dst = em_cur[hi][:, slot]
                            ty = tys[hi]
                            if ty == "C":
                                nc.vector._custom_dve(
                                    EXP_OP, out=dst, in0=s[hi],
                                    in1=mask_sb[:, c, :],
                                    s0=POLY_B[0], s1=POLY_B[1], imm2=POLY_B[2],
                                )
                            elif ty == "Bd":
                                nc.scalar.activation(
                                    dst, s[hi], AF.Exp, bias=negb[:], scale=0.125
                                )
                            else:
                                nc.scalar.activation(
                                    dst, s[hi], AF.Exp, bias=0.0, scale=0.125
                                )
                                nc.gpsimd.tensor_mul(dst, dst,
                                                     mask_sb[:, c, :])
                        if slot == 1:
                            pend.append((pair, list(em_cur)))
                            if len(pend) > 2:
                                emit_av(*pend.pop(0))
                    for item in pend:
                        emit_av(*item)

                    nc.scalar.copy(nh[ha], av[0])
                    nc.vector.tensor_copy(nh[hb], av[1])

                for h in (2, 3):
                    for q in range(4):
                        epilogue_head(h, q)
    nc.finalize()
    return nc


def _to_f8(x):
    import ml_dtypes
    return np.ascontiguousarray(np.asarray(x, dtype=np.float32).astype(
        ml_dtypes.float8_e4m3fn))


def _to_bf16(x):
    import ml_dtypes
    return np.ascontiguousarray(np.asarray(x, dtype=np.float32).astype(
        ml_dtypes.bfloat16))


def _dr_c_layout(xT):
    return np.ascontiguousarray(xT.reshape(2, 128, -1).transpose(1, 0, 2))


def _prep_shared(x_K, Wq, Wk, Wv, Wg, bg, Wo, bo):
    xkt = x_K.T
    xkt8 = _to_f8(_dr_c_layout(xkt))
    wqTb = _to_bf16(Wq.reshape(H, 2, 128, A).transpose(3, 1, 0, 2))
    wkTb = _to_bf16(Wk.reshape(H, 2, 128, A).transpose(3, 1, 0, 2))
    arr = np.empty((128, 2, H * A), np.float32)
    for h in range(H):
        arr[:, :, h * A:(h + 1) * A] = Wv[h].reshape(2, 128, A).transpose(1, 0, 2)
    wv8 = _to_f8(arr)
    wgtb = _to_bf16(Wg.T.reshape(2, 128, H).transpose(1, 0, 2))
    i80d = np.zeros((128, 2, 128), np.float32)
    for p in range(128):
        i80d[p, :, p] = 80.0
    return {
        "xkt8": xkt8, "wqTb": wqTb, "wkTb": wkTb, "wv8": wv8, "wgtb": wgtb,
        "bg": np.asarray(bg, np.float32).reshape(H, 1),
        "wo": np.ascontiguousarray(np.asarray(Wo, np.float32)),
        "bo": np.asarray(bo, np.float32).reshape(1, DO),
        "i80d": _to_f8(i80d),
        "i4": np.eye(H, dtype=np.float32),
    }


def _prep_mask_core(mask_sl):
    import ml_dtypes
    mt = mask_sl.T.astype(np.float32)
    m3 = mt.reshape(NKC, KC, NQC)
    return np.ascontiguousarray(m3.astype(ml_dtypes.float8_e4m3fn))


def kernel(x_Q, x_K, mask, Wq, Wk, Wv, Wg, bg, Wo, bo):
    from concourse.bass_utils import run_bass_kernel_spmd

    x_Q = np.asarray(x_Q, dtype=np.float32)
    x_K = np.asarray(x_K, dtype=np.float32)
    mask = np.asarray(mask, dtype=np.int32)

    shared = _prep_shared(
        x_K, np.asarray(Wq, np.float32), np.asarray(Wk, np.float32),
        np.asarray(Wv, np.float32), np.asarray(Wg, np.float32),
        bg, Wo, bo,
    )

    in_maps = []
    for cidx in range(NCORES):
        sl = slice(cidx * NQC, (cidx + 1) * NQC)
        xqt = x_Q[sl].T
        m = {
            "xqtb": _to_bf16(_dr_c_layout(xqt)),
            "maskx": _prep_mask_core(mask[sl]),
        }
        m.update(shared)
        in_maps.append(m)

    if "nc" not in _cache:
        _cache["nc"] = _build_kernel()
    res = run_bass_kernel_spmd(
        _cache["nc"], in_maps, list(range(NCORES)),
        trace=bool(int(os.environ.get("BASS_KERNEL_TRACE", "0"))),
    )
    if res.exec_time_ns is not None:
        print(f"HW exec time: {res.exec_time_ns} ns")
    return np.concatenate([r["out"] for r in res.results], axis=0)
